# revision 22
# baseline (speedup 1.0000x reference)
"""DoRA linear kernel for 8 Trainium2 NeuronCores.

out = (base_output + 2.0 * x @ lora_A^T @ lora_B^T) * magnitude / (||base_weight + 2.0 * lora_B @ lora_A||_row + eps)

Sharding (row-parallel hint):
  - tokens (B*S = 8192) data-parallel: 1024 per core (x, base_output, out)
  - base_weight / magnitude row-parallel: 512 out_features per core; the
    per-row norm is fully local, mag_scale is allgathered (16KB collective)
  - lora_A / lora_B replicated

Key design points (all layout transforms done on host):
  - x shipped TRANSPOSED (d-major): stage 1 (xa = 2A @ x^T) needs no PE
    transposes.
  - base_output / out transposed (out_features on partitions): the mag
    rescale is a per-partition DVE tensor_scalar, and the base add costs
    ZERO engine cycles -- a gpsimd software-DGE DMA with accum_op=add
    accumulates base^T straight into the delta tile in SBUF.
  - stage-0 square+rowsum runs on DVE via tensor_tensor_reduce (accum_out),
    keeping ACT free for the epilogue PSUM->SBUF copies.
  - base/out bf16, W fp8-e4m3 scaled by 64 (range fix): 49.5 -> 27.6MB HBM.
  - All tiny-descriptor DMAs eliminated (host pre-tiles magsh; the mag
    collective in/out goes through DVE 32x32 block transposes so every DMA
    runs >= 512B-contiguous descriptors).
  - The collective is triggered as soon as stage 0 drains (~25us) so the
    mag-gated tail (DVE scale + stores) rarely waits.

Engine FIFOs (order == emission order per engine):
  sync : magsh b2s a2 at2 x*8 | maglin | stores*32
  ACT  : W*2 b2f | sqrt | xa copies | comb copies*32
  DVE  : stage0 ttr*16, ss reduce, tail | magb transposes | scale*32
  gpsimd: ident8, cc_in, AllGather | base accum-DMA*32
  PE   : stage0 mm*32, stage1 mm*64, stage2 mm*64
"""

import sys

sys.path.insert(0, "/opt/trn_rl_repo")

import ml_dtypes
import numpy as np

import concourse.bass as bass  # noqa: F401
import concourse.mybir as mybir
import concourse.tile as tile
from concourse import bacc
from concourse.bass_utils import run_bass_kernel_spmd
from concourse.masks import make_identity

N_CORES = 8
T, D, O, R = 8192, 4096, 4096, 64
T_LOC = T // N_CORES  # 1024 tokens per core
O_SH = O // N_CORES  # 512 weight rows per core
SCALING = 2.0
EPS = 1e-8
W_SC = 64.0  # fp8 pre-scale for W (and matching 64x on stage-0 A, mag)
F32 = mybir.dt.float32
BF16 = mybir.dt.bfloat16
FP8 = mybir.dt.float8e4
NP_BF16 = ml_dtypes.bfloat16
NP_FP8 = ml_dtypes.float8_e4m3fn

ACCUM_BASE = False  # add base^T via gpsimd accum-DMA (True) or DVE add (False)
N_OC = O // 128  # 32 global o-chunks (epilogue)
N_OCL = O_SH // 128  # 4 local o-chunks (stage 0)
N_DC = D // 128  # 32 d-chunks (stage 1)
N_XC = 8  # x dma chunks (512 d-rows each)

_CACHE: dict = {}


def _emit(nc, tc, aps):
    xt_d = aps["xt"]  # [8, 128, 4096] bf16  x^T chunks
    bt_d = aps["bt"]  # [32, 128, 1024] bf16 base^T per-oc tiles
    wt_d = aps["wt"]  # [128, 16384] fp8     64*W rows as [128, 4 ocl, 4096]
    a2_d = aps["a2"]  # [64, 4096] bf16      128*A (stage-0 rhs)
    at2_d = aps["at2"]  # [128, 2048] bf16   (2A)^T chunks (stage-1 lhsT)
    b2f_d = aps["b2f"]  # [64, 4096] bf16    B^T full
    b2s_d = aps["b2s"]  # [64, 512] bf16     B^T local o-shard
    mags_d = aps["mags"]  # [128, 4] f32     64*magnitude shard (host-tiled)
    out_d = aps["outT"]  # [32, 128, 1024] bf16 out^T tiles

    import contextlib

    ctx = contextlib.ExitStack()
    with ctx:
        const = ctx.enter_context(tc.tile_pool(name="const", bufs=1))
        combpool = ctx.enter_context(tc.tile_pool(name="combpool", bufs=24))
        sqpool = ctx.enter_context(tc.tile_pool(name="sqpool", bufs=4))
        p512 = ctx.enter_context(tc.tile_pool(name="p512", bufs=6, space="PSUM"))
        pxa = ctx.enter_context(tc.tile_pool(name="pxa", bufs=1, space="PSUM"))
        dram = ctx.enter_context(tc.tile_pool(name="dram", bufs=1, space="DRAM"))

        # ---- phase 0: input DMA triggers
        # sync ring: stage0/1 lora consts, then x^T chunks (8MB)
        b2s_sb = const.tile([64, O_SH], BF16)
        nc.sync.dma_start(b2s_sb[:], b2s_d[:])
        a2_sb = const.tile([64, D], BF16)
        nc.sync.dma_start(a2_sb[:], a2_d[:])
        at2_sb = const.tile([128, N_DC * R], BF16)
        nc.sync.dma_start(at2_sb[:], at2_d[:])
        magsh_sb = const.tile([128, 4], F32)
        nc.sync.dma_start(magsh_sb[:], mags_d[:])
        xt_sb = []
        for g in range(N_XC):
            t = const.tile([128, 4096], BF16, name=f"xt_{g}")
            nc.sync.dma_start(t[:], xt_d[g])
            xt_sb.append(t)

        # scalar ring: only W + b2f (2.5MB; clears before ACT's first square).
        # base^T is NOT preloaded -- it is DMA-accumulated into the epilogue
        # tiles by gpsimd, so no bulk trigger can block a compute queue.
        w_sb = const.tile([128, N_OCL * D], FP8)
        nc.scalar.dma_start(w_sb[:, 0 : 2 * D], wt_d[:, 0 : 2 * D])
        nc.scalar.dma_start(w_sb[:, 2 * D : 4 * D], wt_d[:, 2 * D : 4 * D])
        b2f_sb = const.tile([64, O], BF16)
        nc.scalar.dma_start(b2f_sb[:], b2f_d[:])

        # gpsimd: scratch for PE warm-up, identity for the W adds
        wu_sb = const.tile([128, 512], FP8)
        nc.gpsimd.memset(wu_sb[:], 0.25)
        ident8 = const.tile([128, 128], FP8)
        make_identity(nc, ident8[:])

        # ---- PE warm-up: ~20 junk matmuls ramp the tensor engine out of its
        # low p-state before stage 0's real work (result is overwritten by
        # stage 1's first accumulation into the same psum tile)
        pxa_t = pxa.tile([64, 1024], F32, name="pxa01")
        pxa0 = pxa_t[:, 0:512]
        pxa1 = pxa_t[:, 512:1024]
        for wu in range(20):
            nc.tensor.matmul(
                pxa0, ident8[:, 0:64], wu_sb[:], start=True, stop=True
            )

        # ---- stage 0: ss = ||64*(W + 2BA)||^2 per local row -> mag_scale
        # [128,512] psum tiles, drained by ACT Square+accumulator (22) and a
        # DVE bounce/square/reduce pipeline (10)
        ss_sb = const.tile([128, 32], F32)
        for ocl in range(N_OCL):
            for dc in range(8):
                pu = p512.tile([128, 512], F32, tag="ps", name=f"pu_{ocl}_{dc}")
                nc.tensor.matmul(
                    pu[:],
                    b2s_sb[:, 128 * ocl : 128 * (ocl + 1)],
                    a2_sb[:, 512 * dc : 512 * (dc + 1)],
                    start=True,
                    stop=False,
                )
                nc.tensor.matmul(
                    pu[:],
                    ident8[:],
                    w_sb[:, D * ocl + 512 * dc : D * ocl + 512 * (dc + 1)],
                    start=False,
                    stop=True,
                )
                k = 8 * ocl + dc
                if k >= 22:
                    sq = sqpool.tile([128, 512], BF16, tag="sq", name=f"sq_{k}")
                    nc.vector.tensor_scalar_mul(sq[:], pu[:], 1.0)
                    sq2 = sqpool.tile([128, 512], BF16, tag="sq2", name=f"sq2_{k}")
                    nc.vector.tensor_tensor(
                        out=sq2[:], in0=sq[:], in1=sq[:], op=mybir.AluOpType.mult
                    )
                    nc.vector.tensor_reduce(
                        ss_sb[:, k : k + 1],
                        sq2[:],
                        axis=mybir.AxisListType.X,
                        op=mybir.AluOpType.add,
                    )
                else:
                    sq = sqpool.tile([128, 512], BF16, tag="sq", name=f"sq_{k}")
                    nc.scalar.activation(
                        sq[:],
                        pu[:],
                        mybir.ActivationFunctionType.Square,
                        accum_out=ss_sb[:, k : k + 1],
                    )

        # tail: magsc = (64*mag) / (sqrt(ss) + 64*eps), then allgather
        ssr_sb = const.tile([128, N_OCL], F32)
        for ocl in range(N_OCL):
            nc.vector.tensor_reduce(
                ssr_sb[:, ocl : ocl + 1],
                ss_sb[:, 8 * ocl : 8 * (ocl + 1)],
                axis=mybir.AxisListType.X,
                op=mybir.AluOpType.add,
            )
        nrm_sb = const.tile([128, N_OCL], F32)
        nc.scalar.sqrt(nrm_sb[:], ssr_sb[:])
        nc.vector.tensor_scalar_add(nrm_sb[:], nrm_sb[:], W_SC * EPS)
        rinv_sb = const.tile([128, N_OCL], F32)
        nc.vector.reciprocal(rinv_sb[:], nrm_sb[:])
        magsc_sb = const.tile([128, N_OCL], F32)
        nc.vector.tensor_tensor(
            out=magsc_sb[:],
            in0=rinv_sb[:],
            in1=magsh_sb[:],
            op=mybir.AluOpType.mult,
        )
        cc_in = dram.tile([O_SH], F32)
        cc_out = dram.tile([O], F32, addr_space="Shared")
        nc.gpsimd.dma_start(cc_in.rearrange("(oc p) -> p oc", p=128), magsc_sb[:])
        nc.gpsimd.collective_compute(
            "AllGather",
            mybir.AluOpType.bypass,
            replica_groups=[list(range(N_CORES))],
            ins=[cc_in[:]],
            outs=[cc_out[:]],
        )
        # [4096] -> [32,128] contiguous load, then block-transpose to [128,32]
        maglin_sb = const.tile([32, 128], F32)
        nc.sync.dma_start(maglin_sb[:], cc_out.rearrange("(q f) -> q f", f=128))
        magb_sb = const.tile([128, N_OC], F32)
        for b in range(4):
            nc.vector.transpose(
                magb_sb[32 * b : 32 * (b + 1), 0:32],
                maglin_sb[0:32, 32 * b : 32 * (b + 1)],
            )

        # ---- stage 1: xa^T[64, 1024] = (2A) @ x^T, accumulated over d
        for g in range(N_XC):
            for j in range(4):
                dc = 4 * g + j
                lhsT = at2_sb[:, R * dc : R * (dc + 1)]
                nc.tensor.matmul(
                    pxa0,
                    lhsT,
                    xt_sb[g][:, 1024 * j : 1024 * j + 512],
                    start=(dc == 0),
                    stop=(dc == N_DC - 1),
                )
                nc.tensor.matmul(
                    pxa1,
                    lhsT,
                    xt_sb[g][:, 1024 * j + 512 : 1024 * (j + 1)],
                    start=(dc == 0),
                    stop=(dc == N_DC - 1),
                )
        xaT_sb = const.tile([64, 1024], BF16)
        nc.scalar.copy(xaT_sb[:, 0:512], pxa0)
        nc.scalar.copy(xaT_sb[:, 512:1024], pxa1)

        # ---- stage 2 epilogue, per global o-chunk:
        #   PE: delta^T -> PSUM; drain -> comb bf16 (ACT for oc<24, DVE after
        #   so DVE's mag-gated queue can never stall the PSUM rotation);
        #   gpsimd accum-DMA adds base^T for free; DVE scales; sync stores.
        for oc in range(N_OC):
            lhsT = b2f_sb[:, 128 * oc : 128 * (oc + 1)]
            po0 = p512.tile([128, 512], F32, tag="ps", name=f"po_{oc}_0")
            nc.tensor.matmul(po0[:], lhsT, xaT_sb[:, 0:512], start=True, stop=True)
            po1 = p512.tile([128, 512], F32, tag="ps", name=f"po_{oc}_1")
            nc.tensor.matmul(
                po1[:], lhsT, xaT_sb[:, 512:1024], start=True, stop=True
            )
            comb = combpool.tile([128, 1024], BF16, tag="comb", name=f"comb_{oc}")
            if oc < 24:
                nc.scalar.copy(comb[:, 0:512], po0[:])
                nc.scalar.copy(comb[:, 512:1024], po1[:])
            else:
                nc.vector.tensor_scalar_mul(comb[:, 0:512], po0[:], 1.0)
                nc.vector.tensor_scalar_mul(comb[:, 512:1024], po1[:], 1.0)
            nc.gpsimd.dma_start(comb[:], bt_d[oc], accum_op=mybir.AluOpType.add)
            nc.vector.tensor_scalar_mul(comb[:], comb[:], magb_sb[:, oc : oc + 1])
            nc.sync.dma_start(out_d[oc], comb[:])


def _build():
    nc = bacc.Bacc(
        "TRN2", target_bir_lowering=False, debug=False, num_devices=N_CORES
    )
    aps = {
        "xt": nc.dram_tensor("xt", [N_XC, 128, 4096], BF16, kind="ExternalInput").ap(),
        "bt": nc.dram_tensor("bt", [N_OC, 128, T_LOC], BF16, kind="ExternalInput").ap(),
        "wt": nc.dram_tensor("wt", [128, N_OCL * D], FP8, kind="ExternalInput").ap(),
        "a2": nc.dram_tensor("a2", [R, D], BF16, kind="ExternalInput").ap(),
        "at2": nc.dram_tensor("at2", [128, N_DC * R], BF16, kind="ExternalInput").ap(),
        "b2f": nc.dram_tensor("b2f", [R, O], BF16, kind="ExternalInput").ap(),
        "b2s": nc.dram_tensor("b2s", [R, O_SH], BF16, kind="ExternalInput").ap(),
        "mags": nc.dram_tensor("mags", [128, 4], F32, kind="ExternalInput").ap(),
        "outT": nc.dram_tensor(
            "outT", [N_OC, 128, T_LOC], BF16, kind="ExternalOutput"
        ).ap(),
    }
    with tile.TileContext(nc) as tc:
        _emit(nc, tc, aps)
    nc.compile()
    return nc


def run(inputs: dict, trace: bool = False):
    """Run the SPMD kernel on full inputs; returns (full_output, BassKernelResults)."""
    if "nc" not in _CACHE:
        _CACHE["nc"] = _build()
    nc = _CACHE["nc"]

    x = np.asarray(inputs["x"], dtype=np.float32).reshape(T, D).astype(NP_BF16)
    base = np.asarray(inputs["base_output"], dtype=np.float32).reshape(T, O).astype(
        NP_BF16
    )
    w = np.asarray(inputs["base_weight"], dtype=np.float32)
    a = np.asarray(inputs["lora_A"], dtype=np.float32)
    b = np.asarray(inputs["lora_B"], dtype=np.float32)
    mag = np.asarray(inputs["magnitude"], dtype=np.float32)

    a2 = np.ascontiguousarray((W_SC * SCALING * a).astype(NP_BF16))  # [64, D]
    at2 = (SCALING * a).astype(NP_BF16).T  # [D, 64]
    at2 = np.ascontiguousarray(
        at2.reshape(N_DC, 128, R).transpose(1, 0, 2).reshape(128, N_DC * R)
    )
    b2f = np.ascontiguousarray(b.astype(NP_BF16).T)  # [64, O]

    in_maps = []
    for c in range(N_CORES):
        xs = x[c * T_LOC : (c + 1) * T_LOC]  # [1024, 4096] bf16
        bs = base[c * T_LOC : (c + 1) * T_LOC]
        ws = (W_SC * w[c * O_SH : (c + 1) * O_SH]).astype(NP_FP8)  # [512, 4096]
        in_maps.append(
            {
                "xt": np.ascontiguousarray(
                    xs.T.reshape(N_XC, 4, 128, T_LOC)
                    .transpose(0, 2, 1, 3)
                    .reshape(N_XC, 128, 4096)
                ),
                "bt": np.ascontiguousarray(bs.T.reshape(N_OC, 128, T_LOC)),
                "wt": np.ascontiguousarray(
                    ws.reshape(N_OCL, 128, D).transpose(1, 0, 2).reshape(128, N_OCL * D)
                ),
                "a2": a2,
                "at2": at2,
                "b2f": b2f,
                "b2s": np.ascontiguousarray(b2f[:, c * O_SH : (c + 1) * O_SH]),
                "mags": np.ascontiguousarray(
                    (W_SC * mag[c * O_SH : (c + 1) * O_SH]).reshape(N_OCL, 128).T
                ),
            }
        )

    res = run_bass_kernel_spmd(
        nc, in_maps, core_ids=list(range(N_CORES)), trace=trace
    )
    out = np.empty((T, O), dtype=np.float32)
    for c in range(N_CORES):
        out_t = res.results[c]["outT"].reshape(O, T_LOC).astype(np.float32)
        out[c * T_LOC : (c + 1) * T_LOC] = out_t.T
    return out, res


def kernel(**inputs) -> np.ndarray:
    x = inputs["x"]
    out, _ = run(inputs)
    return out.reshape(x.shape[0], x.shape[1], O).astype(np.float32)


# revision 23
# speedup vs baseline: 1.2581x; 1.2581x over previous
"""DoRA linear kernel for 8 Trainium2 NeuronCores.

out = (base_output + 2.0 * x @ lora_A^T @ lora_B^T) * magnitude / (||base_weight + 2.0 * lora_B @ lora_A||_row + eps)

Sharding (row-parallel hint):
  - tokens (B*S = 8192) data-parallel: 1024 per core (x, base_output, out)
  - base_weight / magnitude row-parallel: 512 out_features per core; the
    per-row norm is fully local, mag_scale is allgathered (16KB collective)
  - lora_A / lora_B replicated

Key design points (all layout transforms done on host):
  - x shipped TRANSPOSED (d-major): stage 1 (xa = 2A @ x^T) needs no PE
    transposes.
  - base_output / out transposed (out_features on partitions): the mag
    rescale is a per-partition DVE tensor_scalar, and the base add costs
    ZERO engine cycles -- a gpsimd software-DGE DMA with accum_op=add
    accumulates base^T straight into the delta tile in SBUF.
  - stage-0 square+rowsum runs on DVE via tensor_tensor_reduce (accum_out),
    keeping ACT free for the epilogue PSUM->SBUF copies.
  - base/out bf16, W fp8-e4m3 scaled by 64 (range fix): 49.5 -> 27.6MB HBM.
  - All tiny-descriptor DMAs eliminated (host pre-tiles magsh; the mag
    collective in/out goes through DVE 32x32 block transposes so every DMA
    runs >= 512B-contiguous descriptors).
  - The collective is triggered as soon as stage 0 drains (~25us) so the
    mag-gated tail (DVE scale + stores) rarely waits.

Engine FIFOs (order == emission order per engine):
  sync : magsh b2s a2 at2 x*8 | maglin | stores*32
  ACT  : W*2 b2f | sqrt | xa copies | comb copies*32
  DVE  : stage0 ttr*16, ss reduce, tail | magb transposes | scale*32
  gpsimd: ident8, cc_in, AllGather | base accum-DMA*32
  PE   : stage0 mm*32, stage1 mm*64, stage2 mm*64
"""

import sys

sys.path.insert(0, "/opt/trn_rl_repo")

import ml_dtypes
import numpy as np

import concourse.bass as bass  # noqa: F401
import concourse.mybir as mybir
import concourse.tile as tile
from concourse import bacc
from concourse.bass_utils import run_bass_kernel_spmd
from concourse.masks import make_identity

N_CORES = 8
T, D, O, R = 8192, 4096, 4096, 64
T_LOC = T // N_CORES  # 1024 tokens per core
O_SH = O // N_CORES  # 512 weight rows per core
SCALING = 2.0
EPS = 1e-8
W_SC = 64.0  # fp8 pre-scale for W (and matching 64x on stage-0 A, mag)
F32 = mybir.dt.float32
BF16 = mybir.dt.bfloat16
FP8 = mybir.dt.float8e4
NP_BF16 = ml_dtypes.bfloat16
NP_FP8 = ml_dtypes.float8_e4m3fn

ACCUM_BASE = False  # add base^T via gpsimd accum-DMA (True) or DVE add (False)
N_OC = O // 128  # 32 global o-chunks (epilogue)
N_OCL = O_SH // 128  # 4 local o-chunks (stage 0)
N_DC = D // 128  # 32 d-chunks (stage 1)
N_XC = 8  # x dma chunks (512 d-rows each)

_CACHE: dict = {}


def _emit(nc, tc, aps):
    xt_d = aps["xt"]  # [8, 128, 4096] bf16  x^T chunks
    bt_d = aps["bt"]  # [32, 128, 1024] bf16 base^T per-oc tiles
    wt_d = aps["wt"]  # [128, 16384] fp8     64*W rows as [128, 4 ocl, 4096]
    a2_d = aps["a2"]  # [64, 4096] bf16      128*A (stage-0 rhs)
    at2_d = aps["at2"]  # [128, 2048] bf16   (2A)^T chunks (stage-1 lhsT)
    b2f_d = aps["b2f"]  # [64, 4096] bf16    B^T full
    b2s_d = aps["b2s"]  # [64, 512] bf16     B^T local o-shard
    mags_d = aps["mags"]  # [128, 4] f32     64*magnitude shard (host-tiled)
    out_d = aps["outT"]  # [32, 128, 1024] bf16 out^T tiles

    import contextlib

    ctx = contextlib.ExitStack()
    with ctx:
        const = ctx.enter_context(tc.tile_pool(name="const", bufs=1))
        combpool = ctx.enter_context(tc.tile_pool(name="combpool", bufs=24))
        sqpool = ctx.enter_context(tc.tile_pool(name="sqpool", bufs=4))
        p512 = ctx.enter_context(tc.tile_pool(name="p512", bufs=6, space="PSUM"))
        pxa = ctx.enter_context(tc.tile_pool(name="pxa", bufs=1, space="PSUM"))
        dram = ctx.enter_context(tc.tile_pool(name="dram", bufs=1, space="DRAM"))

        # ---- phase 0: input DMA triggers
        # sync ring: stage0/1 lora consts, then x^T chunks (8MB)
        b2s_sb = const.tile([64, O_SH], BF16)
        nc.sync.dma_start(b2s_sb[:], b2s_d[:])
        a2_sb = const.tile([64, D], BF16)
        nc.sync.dma_start(a2_sb[:], a2_d[:])
        at2_sb = const.tile([128, N_DC * R], BF16)
        nc.sync.dma_start(at2_sb[:], at2_d[:])
        magsh_sb = const.tile([128, 4], F32)
        nc.sync.dma_start(magsh_sb[:], mags_d[:])
        xt_sb = []
        for g in range(N_XC):
            t = const.tile([128, 4096], BF16, name=f"xt_{g}")
            nc.sync.dma_start(t[:], xt_d[g])
            xt_sb.append(t)
        btl_sb = {}
        for oc in range(26, 32):
            t = const.tile([128, T_LOC], BF16, name=f"btl_{oc}")
            nc.sync.dma_start(t[:], bt_d[oc])
            btl_sb[oc] = t

        # scalar ring: only W + b2f (2.5MB; clears before ACT's first square).
        # base^T is NOT preloaded -- it is DMA-accumulated into the epilogue
        # tiles by gpsimd, so no bulk trigger can block a compute queue.
        w_sb = const.tile([128, N_OCL * D], FP8)
        nc.scalar.dma_start(w_sb[:, 0 : 2 * D], wt_d[:, 0 : 2 * D])
        nc.scalar.dma_start(w_sb[:, 2 * D : 4 * D], wt_d[:, 2 * D : 4 * D])
        b2f_sb = const.tile([64, O], BF16)
        nc.scalar.dma_start(b2f_sb[:], b2f_d[:])

        # gpsimd: dummy 4B collective absorbs the ~40us CC-stream barrier
        # while everything else streams; then warm-up scratch + identity
        dram0 = dram  # alias for readability
        dummy_in = dram0.tile([8], F32)
        dummy_out = dram0.tile([8 * N_CORES], F32, addr_space="Shared")
        nc.gpsimd.collective_compute(
            "AllGather",
            mybir.AluOpType.bypass,
            replica_groups=[list(range(N_CORES))],
            ins=[dummy_in[:]],
            outs=[dummy_out[:]],
        )
        wu_sb = const.tile([128, 512], FP8)
        nc.gpsimd.memset(wu_sb[:], 0.25)
        ident8 = const.tile([128, 128], FP8)
        make_identity(nc, ident8[:])

        # ---- PE warm-up: ~20 junk matmuls ramp the tensor engine out of its
        # low p-state before stage 0's real work (result is overwritten by
        # stage 1's first accumulation into the same psum tile)
        pxa_t = pxa.tile([64, 1024], F32, name="pxa01")
        pxa0 = pxa_t[:, 0:512]
        pxa1 = pxa_t[:, 512:1024]
        for wu in range(20):
            nc.tensor.matmul(
                pxa0, ident8[:, 0:64], wu_sb[:], start=True, stop=True
            )

        # ---- stage 0 + stage 1, interleaved on PE so neither input
        # stream (W for the norm, x^T for xa) stalls the tensor engine.
        # stage-0 drains are split ACT (Square+accum) / DVE (bounce+sq+reduce).
        ss_sb = const.tile([128, 32], F32)
        xaT_sb = const.tile([64, 1024], BF16)

        def emit_s0(ocl):
            for dc in range(8):
                pu = p512.tile([128, 512], F32, tag="ps", name=f"pu_{ocl}_{dc}")
                nc.tensor.matmul(
                    pu[:],
                    b2s_sb[:, 128 * ocl : 128 * (ocl + 1)],
                    a2_sb[:, 512 * dc : 512 * (dc + 1)],
                    start=True,
                    stop=False,
                )
                nc.tensor.matmul(
                    pu[:],
                    ident8[:],
                    w_sb[:, D * ocl + 512 * dc : D * ocl + 512 * (dc + 1)],
                    start=False,
                    stop=True,
                )
                k = 8 * ocl + dc
                if k % 2 == 1:
                    sq = sqpool.tile([128, 512], BF16, tag="sq", name=f"sq_{k}")
                    nc.vector.tensor_scalar_mul(sq[:], pu[:], 1.0)
                    sq2 = sqpool.tile([128, 512], BF16, tag="sq2", name=f"sq2_{k}")
                    nc.vector.tensor_tensor(
                        out=sq2[:], in0=sq[:], in1=sq[:], op=mybir.AluOpType.mult
                    )
                    nc.vector.tensor_reduce(
                        ss_sb[:, k : k + 1],
                        sq2[:],
                        axis=mybir.AxisListType.X,
                        op=mybir.AluOpType.add,
                    )
                else:
                    sq = sqpool.tile([128, 512], BF16, tag="sq", name=f"sq_{k}")
                    nc.scalar.activation(
                        sq[:],
                        pu[:],
                        mybir.ActivationFunctionType.Square,
                        accum_out=ss_sb[:, k : k + 1],
                    )

        def emit_s1(g):
            for j in range(4):
                dc = 4 * g + j
                lhsT = at2_sb[:, R * dc : R * (dc + 1)]
                nc.tensor.matmul(
                    pxa0,
                    lhsT,
                    xt_sb[g][:, 1024 * j : 1024 * j + 512],
                    start=(dc == 0),
                    stop=(dc == N_DC - 1),
                )
                nc.tensor.matmul(
                    pxa1,
                    lhsT,
                    xt_sb[g][:, 1024 * j + 512 : 1024 * (j + 1)],
                    start=(dc == 0),
                    stop=(dc == N_DC - 1),
                )

        emit_s1(0)
        for ocl in range(N_OCL):
            emit_s0(ocl)
            emit_s1(1 + ocl)
        for g in range(5, N_XC):
            emit_s1(g)
        nc.scalar.copy(xaT_sb[:, 0:512], pxa0)
        nc.scalar.copy(xaT_sb[:, 512:1024], pxa1)

        # tail: magsc = (64*mag) / (sqrt(ss) + 64*eps), then allgather
        ssr_sb = const.tile([128, N_OCL], F32)
        for ocl in range(N_OCL):
            nc.vector.tensor_reduce(
                ssr_sb[:, ocl : ocl + 1],
                ss_sb[:, 8 * ocl : 8 * (ocl + 1)],
                axis=mybir.AxisListType.X,
                op=mybir.AluOpType.add,
            )
        nrm_sb = const.tile([128, N_OCL], F32)
        nc.scalar.sqrt(nrm_sb[:], ssr_sb[:])
        nc.vector.tensor_scalar_add(nrm_sb[:], nrm_sb[:], W_SC * EPS)
        rinv_sb = const.tile([128, N_OCL], F32)
        nc.vector.reciprocal(rinv_sb[:], nrm_sb[:])
        magsc_sb = const.tile([128, N_OCL], F32)
        nc.vector.tensor_tensor(
            out=magsc_sb[:],
            in0=rinv_sb[:],
            in1=magsh_sb[:],
            op=mybir.AluOpType.mult,
        )
        cc_in = dram.tile([O_SH], F32)
        cc_out = dram.tile([O], F32, addr_space="Shared")
        nc.gpsimd.dma_start(cc_in.rearrange("(oc p) -> p oc", p=128), magsc_sb[:])
        nc.gpsimd.collective_compute(
            "AllGather",
            mybir.AluOpType.bypass,
            replica_groups=[list(range(N_CORES))],
            ins=[cc_in[:]],
            outs=[cc_out[:]],
        )
        # [4096] -> [32,128] contiguous load, then block-transpose to [128,32]
        maglin_sb = const.tile([32, 128], F32)
        nc.sync.dma_start(maglin_sb[:], cc_out.rearrange("(q f) -> q f", f=128))
        magb_sb = const.tile([128, N_OC], F32)
        for b in range(4):
            nc.vector.transpose(
                magb_sb[32 * b : 32 * (b + 1), 0:32],
                maglin_sb[0:32, 32 * b : 32 * (b + 1)],
            )

        # ---- stage 2 epilogue, per global o-chunk:
        #   PE: delta^T -> PSUM
        #   oc < 26 : ACT copies -> comb; gpsimd accum-DMA adds base^T free
        #   oc >= 26: DVE adds base^T straight from PSUM (preloaded tiles)
        #   then DVE per-partition mag scale, sync ring stores
        for oc in range(N_OC):
            lhsT = b2f_sb[:, 128 * oc : 128 * (oc + 1)]
            po0 = p512.tile([128, 512], F32, tag="ps", name=f"po_{oc}_0")
            nc.tensor.matmul(po0[:], lhsT, xaT_sb[:, 0:512], start=True, stop=True)
            po1 = p512.tile([128, 512], F32, tag="ps", name=f"po_{oc}_1")
            nc.tensor.matmul(
                po1[:], lhsT, xaT_sb[:, 512:1024], start=True, stop=True
            )
            comb = combpool.tile([128, 1024], BF16, tag="comb", name=f"comb_{oc}")
            if oc < 26:
                nc.scalar.copy(comb[:, 0:512], po0[:])
                nc.scalar.copy(comb[:, 512:1024], po1[:])
                nc.gpsimd.dma_start(
                    comb[:], bt_d[oc], accum_op=mybir.AluOpType.add
                )
            else:
                bt = btl_sb[oc]
                nc.vector.tensor_tensor(
                    out=comb[:, 0:512], in0=po0[:], in1=bt[:, 0:512],
                    op=mybir.AluOpType.add,
                )
                nc.vector.tensor_tensor(
                    out=comb[:, 512:1024], in0=po1[:], in1=bt[:, 512:1024],
                    op=mybir.AluOpType.add,
                )
            nc.vector.tensor_scalar_mul(comb[:], comb[:], magb_sb[:, oc : oc + 1])
            nc.sync.dma_start(out_d[oc], comb[:])


def _build():
    nc = bacc.Bacc(
        "TRN2", target_bir_lowering=False, debug=False, num_devices=N_CORES
    )
    aps = {
        "xt": nc.dram_tensor("xt", [N_XC, 128, 4096], BF16, kind="ExternalInput").ap(),
        "bt": nc.dram_tensor("bt", [N_OC, 128, T_LOC], BF16, kind="ExternalInput").ap(),
        "wt": nc.dram_tensor("wt", [128, N_OCL * D], FP8, kind="ExternalInput").ap(),
        "a2": nc.dram_tensor("a2", [R, D], BF16, kind="ExternalInput").ap(),
        "at2": nc.dram_tensor("at2", [128, N_DC * R], BF16, kind="ExternalInput").ap(),
        "b2f": nc.dram_tensor("b2f", [R, O], BF16, kind="ExternalInput").ap(),
        "b2s": nc.dram_tensor("b2s", [R, O_SH], BF16, kind="ExternalInput").ap(),
        "mags": nc.dram_tensor("mags", [128, 4], F32, kind="ExternalInput").ap(),
        "outT": nc.dram_tensor(
            "outT", [N_OC, 128, T_LOC], BF16, kind="ExternalOutput"
        ).ap(),
    }
    with tile.TileContext(nc) as tc:
        _emit(nc, tc, aps)
    nc.compile()
    return nc


def run(inputs: dict, trace: bool = False):
    """Run the SPMD kernel on full inputs; returns (full_output, BassKernelResults)."""
    if "nc" not in _CACHE:
        _CACHE["nc"] = _build()
    nc = _CACHE["nc"]

    x = np.asarray(inputs["x"], dtype=np.float32).reshape(T, D).astype(NP_BF16)
    base = np.asarray(inputs["base_output"], dtype=np.float32).reshape(T, O).astype(
        NP_BF16
    )
    w = np.asarray(inputs["base_weight"], dtype=np.float32)
    a = np.asarray(inputs["lora_A"], dtype=np.float32)
    b = np.asarray(inputs["lora_B"], dtype=np.float32)
    mag = np.asarray(inputs["magnitude"], dtype=np.float32)

    a2 = np.ascontiguousarray((W_SC * SCALING * a).astype(NP_BF16))  # [64, D]
    at2 = (SCALING * a).astype(NP_BF16).T  # [D, 64]
    at2 = np.ascontiguousarray(
        at2.reshape(N_DC, 128, R).transpose(1, 0, 2).reshape(128, N_DC * R)
    )
    b2f = np.ascontiguousarray(b.astype(NP_BF16).T)  # [64, O]

    in_maps = []
    for c in range(N_CORES):
        xs = x[c * T_LOC : (c + 1) * T_LOC]  # [1024, 4096] bf16
        bs = base[c * T_LOC : (c + 1) * T_LOC]
        ws = (W_SC * w[c * O_SH : (c + 1) * O_SH]).astype(NP_FP8)  # [512, 4096]
        in_maps.append(
            {
                "xt": np.ascontiguousarray(
                    xs.T.reshape(N_XC, 4, 128, T_LOC)
                    .transpose(0, 2, 1, 3)
                    .reshape(N_XC, 128, 4096)
                ),
                "bt": np.ascontiguousarray(bs.T.reshape(N_OC, 128, T_LOC)),
                "wt": np.ascontiguousarray(
                    ws.reshape(N_OCL, 128, D).transpose(1, 0, 2).reshape(128, N_OCL * D)
                ),
                "a2": a2,
                "at2": at2,
                "b2f": b2f,
                "b2s": np.ascontiguousarray(b2f[:, c * O_SH : (c + 1) * O_SH]),
                "mags": np.ascontiguousarray(
                    (W_SC * mag[c * O_SH : (c + 1) * O_SH]).reshape(N_OCL, 128).T
                ),
            }
        )

    res = run_bass_kernel_spmd(
        nc, in_maps, core_ids=list(range(N_CORES)), trace=trace
    )
    out = np.empty((T, O), dtype=np.float32)
    for c in range(N_CORES):
        out_t = res.results[c]["outT"].reshape(O, T_LOC).astype(np.float32)
        out[c * T_LOC : (c + 1) * T_LOC] = out_t.T
    return out, res


def kernel(**inputs) -> np.ndarray:
    x = inputs["x"]
    out, _ = run(inputs)
    return out.reshape(x.shape[0], x.shape[1], O).astype(np.float32)


# revision 24
# speedup vs baseline: 1.3267x; 1.0545x over previous
"""DoRA linear kernel for 8 Trainium2 NeuronCores.

out = (base_output + 2.0 * x @ lora_A^T @ lora_B^T) * magnitude / (||base_weight + 2.0 * lora_B @ lora_A||_row + eps)

Sharding (row-parallel hint):
  - tokens (B*S = 8192) data-parallel: 1024 per core (x, base_output, out)
  - base_weight / magnitude row-parallel: 512 out_features per core; the
    per-row norm is fully local, mag_scale is allgathered (16KB collective)
  - lora_A / lora_B replicated

Key design points (all layout transforms done on host):
  - x shipped TRANSPOSED (d-major): stage 1 (xa = 2A @ x^T) needs no PE
    transposes.
  - base_output / out transposed (out_features on partitions): the mag
    rescale is a per-partition DVE tensor_scalar, and the base add costs
    ZERO engine cycles -- a gpsimd software-DGE DMA with accum_op=add
    accumulates base^T straight into the delta tile in SBUF.
  - stage-0 square+rowsum runs on DVE via tensor_tensor_reduce (accum_out),
    keeping ACT free for the epilogue PSUM->SBUF copies.
  - base/out bf16, W fp8-e4m3 scaled by 64 (range fix): 49.5 -> 27.6MB HBM.
  - All tiny-descriptor DMAs eliminated (host pre-tiles magsh; the mag
    collective in/out goes through DVE 32x32 block transposes so every DMA
    runs >= 512B-contiguous descriptors).
  - The collective is triggered as soon as stage 0 drains (~25us) so the
    mag-gated tail (DVE scale + stores) rarely waits.

Engine FIFOs (order == emission order per engine):
  sync : magsh b2s a2 at2 x*8 | maglin | stores*32
  ACT  : W*2 b2f | sqrt | xa copies | comb copies*32
  DVE  : stage0 ttr*16, ss reduce, tail | magb transposes | scale*32
  gpsimd: ident8, cc_in, AllGather | base accum-DMA*32
  PE   : stage0 mm*32, stage1 mm*64, stage2 mm*64
"""

import sys

sys.path.insert(0, "/opt/trn_rl_repo")

import ml_dtypes
import numpy as np

import concourse.bass as bass  # noqa: F401
import concourse.mybir as mybir
import concourse.tile as tile
from concourse import bacc
from concourse.bass_utils import run_bass_kernel_spmd
from concourse.masks import make_identity

N_CORES = 8
T, D, O, R = 8192, 4096, 4096, 64
T_LOC = T // N_CORES  # 1024 tokens per core
O_SH = O // N_CORES  # 512 weight rows per core
SCALING = 2.0
EPS = 1e-8
W_SC = 64.0  # fp8 pre-scale for W (and matching 64x on stage-0 A, mag)
F32 = mybir.dt.float32
BF16 = mybir.dt.bfloat16
FP8 = mybir.dt.float8e4
NP_BF16 = ml_dtypes.bfloat16
NP_FP8 = ml_dtypes.float8_e4m3fn

ACCUM_BASE = False  # add base^T via gpsimd accum-DMA (True) or DVE add (False)
N_OC = O // 128  # 32 global o-chunks (epilogue)
N_OCL = O_SH // 128  # 4 local o-chunks (stage 0)
N_DC = D // 128  # 32 d-chunks (stage 1)
N_XC = 8  # x dma chunks (512 d-rows each)

_CACHE: dict = {}


def _emit(nc, tc, aps):
    xt_d = aps["xt"]  # [8, 128, 4096] bf16  x^T chunks
    bt_d = aps["bt"]  # [32, 128, 1024] bf16 base^T per-oc tiles
    wt_d = aps["wt"]  # [128, 16384] fp8     64*W rows as [128, 4 ocl, 4096]
    a2_d = aps["a2"]  # [64, 4096] bf16      128*A (stage-0 rhs)
    at2_d = aps["at2"]  # [128, 2048] bf16   (2A)^T chunks (stage-1 lhsT)
    b2f_d = aps["b2f"]  # [64, 4096] bf16    B^T full
    b2s_d = aps["b2s"]  # [64, 512] bf16     B^T local o-shard
    mags_d = aps["mags"]  # [128, 4] f32     64*magnitude shard (host-tiled)
    out_d = aps["outT"]  # [32, 128, 1024] bf16 out^T tiles

    import contextlib

    ctx = contextlib.ExitStack()
    with ctx:
        const = ctx.enter_context(tc.tile_pool(name="const", bufs=1))
        combpool = ctx.enter_context(tc.tile_pool(name="combpool", bufs=28))
        sqpool = ctx.enter_context(tc.tile_pool(name="sqpool", bufs=4))
        p512 = ctx.enter_context(tc.tile_pool(name="p512", bufs=6, space="PSUM"))
        pxa = ctx.enter_context(tc.tile_pool(name="pxa", bufs=1, space="PSUM"))
        dram = ctx.enter_context(tc.tile_pool(name="dram", bufs=1, space="DRAM"))

        # ---- phase 0: input DMA triggers
        # sync ring: stage0/1 lora consts, then x^T chunks (8MB)
        b2s_sb = const.tile([64, O_SH], BF16)
        nc.sync.dma_start(b2s_sb[:], b2s_d[:])
        a2_sb = const.tile([64, D], BF16)
        nc.sync.dma_start(a2_sb[:], a2_d[:])
        at2_sb = const.tile([128, N_DC * R], BF16)
        nc.sync.dma_start(at2_sb[:], at2_d[:])
        magsh_sb = const.tile([128, 4], F32)
        nc.sync.dma_start(magsh_sb[:], mags_d[:])
        xt_sb = []
        for g in range(N_XC):
            t = const.tile([128, 4096], BF16, name=f"xt_{g}")
            nc.sync.dma_start(t[:], xt_d[g])
            xt_sb.append(t)
        btl_sb = {}
        for oc in range(26, 32):
            t = const.tile([128, T_LOC], BF16, name=f"btl_{oc}")
            nc.sync.dma_start(t[:], bt_d[oc])
            btl_sb[oc] = t

        # scalar ring: only W + b2f (2.5MB; clears before ACT's first square).
        # base^T is NOT preloaded -- it is DMA-accumulated into the epilogue
        # tiles by gpsimd, so no bulk trigger can block a compute queue.
        w_sb = const.tile([128, N_OCL * D], FP8)
        nc.scalar.dma_start(w_sb[:, 0 : 2 * D], wt_d[:, 0 : 2 * D])
        nc.scalar.dma_start(w_sb[:, 2 * D : 4 * D], wt_d[:, 2 * D : 4 * D])
        b2f_sb = const.tile([64, O], BF16)
        nc.scalar.dma_start(b2f_sb[:], b2f_d[:])

        # identities are host-shipped (no gpsimd affine_select on the
        # critical path); ident8 feeds the stage-0 W adds
        ident8 = const.tile([128, 128], FP8)
        nc.scalar.dma_start(ident8[:], aps["id8"][:])
        identb = const.tile([128, 128], BF16)
        nc.sync.dma_start(identb[:], aps["idb"][:])

        # ---- PE warm-up: ~20 junk matmuls ramp the tensor engine out of its
        # low p-state before stage 0's real work (result is overwritten by
        # stage 1's first accumulation into the same psum tile)
        pxa_t = pxa.tile([64, 1024], F32, name="pxa01")
        pxa0 = pxa_t[:, 0:512]
        pxa1 = pxa_t[:, 512:1024]
        for wu in range(20):
            nc.tensor.matmul(
                pxa0, identb[:, 0:64], at2_sb[:, 0:512], start=True, stop=True
            )

        # ---- stage 0 + stage 1, interleaved on PE so neither input
        # stream (W for the norm, x^T for xa) stalls the tensor engine.
        # stage-0 drains are split ACT (Square+accum) / DVE (bounce+sq+reduce).
        ss_sb = const.tile([128, 32], F32)
        xaT_sb = const.tile([64, 1024], BF16)

        def emit_s0(ocl):
            for dc in range(8):
                pu = p512.tile([128, 512], F32, tag="ps", name=f"pu_{ocl}_{dc}")
                nc.tensor.matmul(
                    pu[:],
                    b2s_sb[:, 128 * ocl : 128 * (ocl + 1)],
                    a2_sb[:, 512 * dc : 512 * (dc + 1)],
                    start=True,
                    stop=False,
                )
                nc.tensor.matmul(
                    pu[:],
                    ident8[:],
                    w_sb[:, D * ocl + 512 * dc : D * ocl + 512 * (dc + 1)],
                    start=False,
                    stop=True,
                )
                k = 8 * ocl + dc
                if k % 3 == 2:
                    sq = sqpool.tile([128, 512], BF16, tag="sq", name=f"sq_{k}")
                    nc.vector.tensor_scalar_mul(sq[:], pu[:], 1.0)
                    sq2 = sqpool.tile([128, 512], BF16, tag="sq2", name=f"sq2_{k}")
                    nc.vector.tensor_tensor(
                        out=sq2[:], in0=sq[:], in1=sq[:], op=mybir.AluOpType.mult
                    )
                    nc.vector.tensor_reduce(
                        ss_sb[:, k : k + 1],
                        sq2[:],
                        axis=mybir.AxisListType.X,
                        op=mybir.AluOpType.add,
                    )
                else:
                    sq = sqpool.tile([128, 512], BF16, tag="sq", name=f"sq_{k}")
                    nc.scalar.activation(
                        sq[:],
                        pu[:],
                        mybir.ActivationFunctionType.Square,
                        accum_out=ss_sb[:, k : k + 1],
                    )

        def emit_s1(g):
            for j in range(4):
                dc = 4 * g + j
                lhsT = at2_sb[:, R * dc : R * (dc + 1)]
                nc.tensor.matmul(
                    pxa0,
                    lhsT,
                    xt_sb[g][:, 1024 * j : 1024 * j + 512],
                    start=(dc == 0),
                    stop=(dc == N_DC - 1),
                )
                nc.tensor.matmul(
                    pxa1,
                    lhsT,
                    xt_sb[g][:, 1024 * j + 512 : 1024 * (j + 1)],
                    start=(dc == 0),
                    stop=(dc == N_DC - 1),
                )

        emit_s1(0)
        for ocl in range(N_OCL):
            emit_s0(ocl)
            emit_s1(1 + ocl)
        for g in range(5, N_XC):
            emit_s1(g)
        nc.scalar.copy(xaT_sb[:, 0:512], pxa0)
        nc.scalar.copy(xaT_sb[:, 512:1024], pxa1)

        # tail: magsc = (64*mag) / (sqrt(ss) + 64*eps), then allgather
        ssr_sb = const.tile([128, N_OCL], F32)
        for ocl in range(N_OCL):
            nc.vector.tensor_reduce(
                ssr_sb[:, ocl : ocl + 1],
                ss_sb[:, 8 * ocl : 8 * (ocl + 1)],
                axis=mybir.AxisListType.X,
                op=mybir.AluOpType.add,
            )
        nrm_sb = const.tile([128, N_OCL], F32)
        nc.scalar.sqrt(nrm_sb[:], ssr_sb[:])
        nc.vector.tensor_scalar_add(nrm_sb[:], nrm_sb[:], W_SC * EPS)
        rinv_sb = const.tile([128, N_OCL], F32)
        nc.vector.reciprocal(rinv_sb[:], nrm_sb[:])
        magsc_sb = const.tile([128, N_OCL], F32)
        nc.vector.tensor_tensor(
            out=magsc_sb[:],
            in0=rinv_sb[:],
            in1=magsh_sb[:],
            op=mybir.AluOpType.mult,
        )
        cc_in = dram.tile([O_SH], F32)
        cc_out = dram.tile([O], F32, addr_space="Shared")
        nc.gpsimd.dma_start(cc_in.rearrange("(oc p) -> p oc", p=128), magsc_sb[:])
        nc.gpsimd.collective_compute(
            "AllGather",
            mybir.AluOpType.bypass,
            replica_groups=[list(range(N_CORES))],
            ins=[cc_in[:]],
            outs=[cc_out[:]],
        )
        # [4096] -> [32,128] contiguous load, then block-transpose to [128,32]
        maglin_sb = const.tile([32, 128], F32)
        nc.sync.dma_start(maglin_sb[:], cc_out.rearrange("(q f) -> q f", f=128))
        magb_sb = const.tile([128, N_OC], F32)
        for b in range(4):
            nc.vector.transpose(
                magb_sb[32 * b : 32 * (b + 1), 0:32],
                maglin_sb[0:32, 32 * b : 32 * (b + 1)],
            )

        # ---- stage 2 epilogue, per global o-chunk:
        #   PE: delta^T -> PSUM
        #   oc < 26 : ACT copies -> comb; gpsimd accum-DMA adds base^T free
        #   oc >= 26: DVE adds base^T straight from PSUM (preloaded tiles)
        #   then DVE per-partition mag scale, sync ring stores
        for oc in range(N_OC):
            lhsT = b2f_sb[:, 128 * oc : 128 * (oc + 1)]
            po0 = p512.tile([128, 512], F32, tag="ps", name=f"po_{oc}_0")
            nc.tensor.matmul(po0[:], lhsT, xaT_sb[:, 0:512], start=True, stop=True)
            po1 = p512.tile([128, 512], F32, tag="ps", name=f"po_{oc}_1")
            nc.tensor.matmul(
                po1[:], lhsT, xaT_sb[:, 512:1024], start=True, stop=True
            )
            comb = combpool.tile([128, 1024], BF16, tag="comb", name=f"comb_{oc}")
            if oc < 26:
                nc.scalar.copy(comb[:, 0:512], po0[:])
                nc.scalar.copy(comb[:, 512:1024], po1[:])
                nc.gpsimd.dma_start(
                    comb[:], bt_d[oc], accum_op=mybir.AluOpType.add
                )
            else:
                bt = btl_sb[oc]
                nc.vector.tensor_tensor(
                    out=comb[:, 0:512], in0=po0[:], in1=bt[:, 0:512],
                    op=mybir.AluOpType.add,
                )
                nc.vector.tensor_tensor(
                    out=comb[:, 512:1024], in0=po1[:], in1=bt[:, 512:1024],
                    op=mybir.AluOpType.add,
                )
            nc.vector.tensor_scalar_mul(comb[:], comb[:], magb_sb[:, oc : oc + 1])
            nc.sync.dma_start(out_d[oc], comb[:])


def _build():
    nc = bacc.Bacc(
        "TRN2", target_bir_lowering=False, debug=False, num_devices=N_CORES
    )
    aps = {
        "xt": nc.dram_tensor("xt", [N_XC, 128, 4096], BF16, kind="ExternalInput").ap(),
        "bt": nc.dram_tensor("bt", [N_OC, 128, T_LOC], BF16, kind="ExternalInput").ap(),
        "wt": nc.dram_tensor("wt", [128, N_OCL * D], FP8, kind="ExternalInput").ap(),
        "a2": nc.dram_tensor("a2", [R, D], BF16, kind="ExternalInput").ap(),
        "at2": nc.dram_tensor("at2", [128, N_DC * R], BF16, kind="ExternalInput").ap(),
        "b2f": nc.dram_tensor("b2f", [R, O], BF16, kind="ExternalInput").ap(),
        "b2s": nc.dram_tensor("b2s", [R, O_SH], BF16, kind="ExternalInput").ap(),
        "mags": nc.dram_tensor("mags", [128, 4], F32, kind="ExternalInput").ap(),
        "id8": nc.dram_tensor("id8", [128, 128], FP8, kind="ExternalInput").ap(),
        "idb": nc.dram_tensor("idb", [128, 128], BF16, kind="ExternalInput").ap(),
        "outT": nc.dram_tensor(
            "outT", [N_OC, 128, T_LOC], BF16, kind="ExternalOutput"
        ).ap(),
    }
    with tile.TileContext(nc) as tc:
        _emit(nc, tc, aps)
    nc.compile()
    return nc


def run(inputs: dict, trace: bool = False):
    """Run the SPMD kernel on full inputs; returns (full_output, BassKernelResults)."""
    if "nc" not in _CACHE:
        _CACHE["nc"] = _build()
    nc = _CACHE["nc"]

    x = np.asarray(inputs["x"], dtype=np.float32).reshape(T, D).astype(NP_BF16)
    base = np.asarray(inputs["base_output"], dtype=np.float32).reshape(T, O).astype(
        NP_BF16
    )
    w = np.asarray(inputs["base_weight"], dtype=np.float32)
    a = np.asarray(inputs["lora_A"], dtype=np.float32)
    b = np.asarray(inputs["lora_B"], dtype=np.float32)
    mag = np.asarray(inputs["magnitude"], dtype=np.float32)

    a2 = np.ascontiguousarray((W_SC * SCALING * a).astype(NP_BF16))  # [64, D]
    at2 = (SCALING * a).astype(NP_BF16).T  # [D, 64]
    at2 = np.ascontiguousarray(
        at2.reshape(N_DC, 128, R).transpose(1, 0, 2).reshape(128, N_DC * R)
    )
    b2f = np.ascontiguousarray(b.astype(NP_BF16).T)  # [64, O]

    in_maps = []
    for c in range(N_CORES):
        xs = x[c * T_LOC : (c + 1) * T_LOC]  # [1024, 4096] bf16
        bs = base[c * T_LOC : (c + 1) * T_LOC]
        ws = (W_SC * w[c * O_SH : (c + 1) * O_SH]).astype(NP_FP8)  # [512, 4096]
        in_maps.append(
            {
                "xt": np.ascontiguousarray(
                    xs.T.reshape(N_XC, 4, 128, T_LOC)
                    .transpose(0, 2, 1, 3)
                    .reshape(N_XC, 128, 4096)
                ),
                "bt": np.ascontiguousarray(bs.T.reshape(N_OC, 128, T_LOC)),
                "wt": np.ascontiguousarray(
                    ws.reshape(N_OCL, 128, D).transpose(1, 0, 2).reshape(128, N_OCL * D)
                ),
                "a2": a2,
                "at2": at2,
                "b2f": b2f,
                "b2s": np.ascontiguousarray(b2f[:, c * O_SH : (c + 1) * O_SH]),
                "mags": np.ascontiguousarray(
                    (W_SC * mag[c * O_SH : (c + 1) * O_SH]).reshape(N_OCL, 128).T
                ),
                "id8": np.eye(128, dtype=NP_FP8),
                "idb": np.eye(128, dtype=NP_BF16),
            }
        )

    res = run_bass_kernel_spmd(
        nc, in_maps, core_ids=list(range(N_CORES)), trace=trace
    )
    out = np.empty((T, O), dtype=np.float32)
    for c in range(N_CORES):
        out_t = res.results[c]["outT"].reshape(O, T_LOC).astype(np.float32)
        out[c * T_LOC : (c + 1) * T_LOC] = out_t.T
    return out, res


def kernel(**inputs) -> np.ndarray:
    x = inputs["x"]
    out, _ = run(inputs)
    return out.reshape(x.shape[0], x.shape[1], O).astype(np.float32)
